# revision 36
# baseline (speedup 1.0000x reference)
"""Trainium2 Bass kernel for nn_MultiHeadAttention_77283641524724.

Gaussian-kernel multi-head attention + residual + custom LayerNorm.

Sharding (8 cores): core c handles batch c//4 and heads [4*(c%4), 4*(c%4)+4).
Each core computes its 4 heads' QKV projections, attention, and its 256-col
slice of the head-concat; LayerNorm (over the full 1024 features) is realized
with a tiny AllReduce of per-row partial (sum, sumsq) within each batch's
4-core group, after which every core normalizes its own feature slice.
Host-side gather is a plain concatenate along the feature axis.

Math notes:
- scores = scale*(q.k - 0.5||q||^2 - 0.5||k||^2); the -0.5||q||^2 term is a
  per-query-row constant and softmax is invariant to it -> dropped.
- scale is folded into Wq/bq on the host.
- -0.5*scale*||k||^2 rides in the score matmul as a 65th contraction row
  (k-side row = norms, q-side row = ones).
- score range is ~[-0.7, 0.4] for this distribution -> exp without
  max-subtraction is safe (reference softmax is shift-invariant).
- softmax denominator comes from a ones-column appended to V (65-col
  stationary operand), so attn@V yields [out | norm] in one accumulation.
- matmul operands are bf16 (fp32 PSUM accumulation); residual add + LN are
  fp32.
"""

import numpy as np
import ml_dtypes

import concourse.bass as bass
import concourse.bacc as bacc
import concourse.tile as tile
from concourse import mybir
import concourse.bass_utils as bass_utils
from concourse.masks import make_identity

BF16 = mybir.dt.bfloat16
F32 = mybir.dt.float32
NPBF16 = ml_dtypes.bfloat16

B, S, E = 2, 2048, 1024
H, DK, DV = 16, 64, 64
EPS = 1e-6
SCALE = 1.0 / float(np.sqrt(np.float32(E)))
N_CORES = 8
HPC = 4            # heads per core
DHC = HPC * DV     # 256 output cols per core
VW = HPC * (DV + 1)  # 260: v + ones col per head
P = 128
NE = E // P        # 8 contraction tiles
NST = S // P       # 16 seq tiles of 128
NSC = S // 512     # 4 seq chunks of 512
NKT = S // P       # 16 key tiles
AF = mybir.ActivationFunctionType

_NC_CACHE = None


def _bcast_ap(ap, p):
    """[1, n] DRAM AP -> [[0, p], [1, n]] partition-broadcast AP."""
    return bass.AP(tensor=ap.tensor, offset=ap.offset, ap=[[0, p], ap.ap[-1]])


def _emit(nc, tc, io, no_collective=False, ln_fast=False):
    from contextlib import ExitStack

    with ExitStack() as ctx:
        consts = ctx.enter_context(tc.tile_pool(name="consts", bufs=1))
        persist = ctx.enter_context(tc.tile_pool(name="persist", bufs=1))
        dram = ctx.enter_context(tc.tile_pool(name="dram", bufs=1, space="DRAM"))

        ident = consts.tile([P, P], F32, tag="ident", name="ident")
        make_identity(nc, ident)
        negcol = consts.tile([DK, 1], BF16, tag="negcol", name="negcol")
        nc.vector.memset(negcol, -0.5 * SCALE)
        # Small consts ride the SWDGE queue so the HWDGE queues start on the
        # big input tensors immediately.
        ones_sb = consts.tile([1, S], BF16, tag="ones", name="ones")
        nc.gpsimd.dma_start(ones_sb, io["ones_row"])
        bq_sb = consts.tile([1, DHC], BF16, tag="bq", name="bq")
        nc.gpsimd.dma_start(bq_sb, io["bq"])
        bk_sb = consts.tile([1, DHC], BF16, tag="bk", name="bk")
        nc.gpsimd.dma_start(bk_sb, io["bk"])
        bv_sb = consts.tile([1, VW], BF16, tag="bv", name="bv")
        nc.gpsimd.dma_start(bv_sb, io["bv"])
        esb = consts.tile([P, DHC], F32, tag="esb", name="esb")
        nc.gpsimd.dma_start(esb, _bcast_ap(io["epsshift"], P))
        lnsc = consts.tile([P, DHC], F32, tag="lnsc", name="lnsc")
        nc.gpsimd.dma_start(lnsc, _bcast_ap(io["lnscale"], P))

        # Persistent per-head / per-seq-tile tensors.
        q_sb = [persist.tile([DK + 1, S], BF16, tag=f"q{h}", name=f"q{h}") for h in range(HPC)]
        k_sb = [persist.tile([DK + 1, S], BF16, tag=f"k{h}", name=f"k{h}") for h in range(HPC)]
        v_sb = [persist.tile([P, VW], BF16, tag=f"v{st}", name=f"v{st}") for st in range(NST)]
        x_sb = [persist.tile([P, DHC], F32, tag=f"x{st}", name=f"x{st}") for st in range(NST)]

        # ------- Stages B (projections) + D (attention), interleaved -------
        # Emission order sets scheduler priority: V; K/Q for heads 0-1;
        # attention heads 0-1; K/Q for heads 2-3 (fills PE while ACT crunches
        # exp); attention heads 2-3.  One shared PSUM pool: "scores" 2x2
        # banks + "small" 4x1 banks (projections / attn accumulators /
        # transposes all fit a [128,512]-f32 slot).
        with (
            tc.tile_pool(name="kqin", bufs=1) as kqin,
            tc.tile_pool(name="psum", bufs=1, space="PSUM") as psum,
            tc.tile_pool(name="sksq", bufs=1) as sksq,
            tc.tile_pool(name="sexp", bufs=8) as sexp,
            tc.tile_pool(name="susb", bufs=4) as susb,
            tc.tile_pool(name="ssml", bufs=8) as ssml,
        ):
            kT_sb = [kqin.tile([P, S], BF16, tag=f"kT{e}", name=f"kT{e}") for e in range(NE)]
            qT_sb = [kqin.tile([P, S], BF16, tag=f"qT{e}", name=f"qT{e}") for e in range(NE)]
            wq_sb = [kqin.tile([P, DHC], BF16, tag=f"wq{e}", name=f"wq{e}") for e in range(NE)]
            wk_sb = [kqin.tile([P, DHC], BF16, tag=f"wk{e}", name=f"wk{e}") for e in range(NE)]

            def proj_kq(w_sb, in_sb, b_row, dst, dt):
                # dst[2dt], dst[2dt+1] rows 0:64 <- [d, s] proj of head pair dt
                dsl = slice(P * dt, P * dt + P)
                for sc in range(NSC):
                    ssl = slice(512 * sc, 512 * sc + 512)
                    ps = psum.tile([P, 512], F32, tag="small", name="proj", bufs=4)
                    for e in range(NE):
                        nc.tensor.matmul(
                            ps, w_sb[e][:, dsl], in_sb[e][:, ssl],
                            start=(e == 0), stop=False,
                        )
                    nc.tensor.matmul(
                        ps, b_row[:, dsl], ones_sb[:, ssl],
                        start=False, stop=True,
                    )
                    nc.vector.tensor_copy(dst[2 * dt][0:DK, ssl], ps[0:DK, :])
                    nc.vector.tensor_copy(dst[2 * dt + 1][0:DK, ssl], ps[DK:P, :])

            def knorms(h):
                # k_sb[h] row 64 <- -0.5*scale*||k||^2 per key.
                ksq = sksq.tile([DK, S], BF16, tag="ksq", name="ksq")
                nc.vector.tensor_mul(ksq, k_sb[h][0:DK, :], k_sb[h][0:DK, :])
                for sc in range(NSC):
                    ssl = slice(512 * sc, 512 * sc + 512)
                    pn = psum.tile([1, 512], F32, tag="small", name="pn", bufs=4)
                    nc.tensor.matmul(pn, negcol, ksq[:, ssl], start=True, stop=True)
                    nc.vector.tensor_copy(k_sb[h][DK:DK + 1, ssl], pn)

            def attn_head(h):
                vsl = slice((DV + 1) * h, (DV + 1) * (h + 1))
                for qh in range(2):  # query halves of 1024
                    avs = [psum.tile([DV + 1, 512], F32, tag="small", name="av", bufs=4)
                           for _ in range(2)]
                    for kt in range(NKT):
                        ksl = slice(P * kt, P * kt + P)
                        sc_ps = psum.tile([P, 1024], F32, tag="scores", name="scores", bufs=2)
                        for qq in range(2):
                            qc = 2 * qh + qq
                            qsl = slice(512 * qc, 512 * qc + 512)
                            nc.tensor.matmul(
                                sc_ps[:, 512 * qq:512 * qq + 512],
                                k_sb[h][:, ksl], q_sb[h][:, qsl],
                                start=True, stop=True,
                            )
                        e_sb = sexp.tile([P, 1024], BF16, tag="exp", name="exp")
                        nc.scalar.activation(e_sb, sc_ps, AF.Exp, bias=0.0, scale=1.0)
                        for qq in range(2):
                            nc.tensor.matmul(
                                avs[qq], v_sb[kt][:, vsl],
                                e_sb[:, 512 * qq:512 * qq + 512],
                                start=(kt == 0), stop=(kt == NKT - 1),
                            )
                    for qq in range(2):
                        u = susb.tile([DV + 1, 512], F32, tag="usb", name="usb")
                        nc.vector.tensor_copy(u, avs[qq])
                        for pi in range(4):
                            st = 8 * qh + 4 * qq + pi
                            tp = psum.tile([P, DV + 1], F32, tag="small", name="tp", bufs=4)
                            nc.tensor.transpose(
                                tp, u[:, P * pi:P * pi + P],
                                ident[0:DV + 1, 0:DV + 1],
                            )
                            rec = ssml.tile([P, 1], F32, tag="rec", name="rec")
                            nc.vector.reciprocal(rec, tp[:, DV:DV + 1])
                            nc.vector.tensor_scalar_mul(
                                x_sb[st][:, DV * h:DV * h + DV], tp[:, 0:DV], rec
                            )

            with tc.tile_pool(name="vin", bufs=1) as vin:
                vT_sb = [vin.tile([P, S], BF16, tag=f"vT{e}", name=f"vT{e}") for e in range(NE)]
                wv_sb = [vin.tile([P, VW], BF16, tag=f"wv{e}", name=f"wv{e}") for e in range(NE)]
                # Three DMA queues in parallel: SP=wv/vT, ACT=wk/kT,
                # SWDGE=wq/qT.  K/Q gate the first scores; V matmuls are
                # pulled in by the scheduler per k-tile as attention needs
                # them.
                for e in range(NE):
                    sl = slice(P * e, P * e + P)
                    nc.sync.dma_start(wq_sb[e], io["wq"][sl, :])
                    nc.sync.dma_start(qT_sb[e], io["qT"][sl, :])
                    nc.scalar.dma_start(wk_sb[e], io["wk"][sl, :])
                    nc.scalar.dma_start(kT_sb[e], io["kT"][sl, :])
                    nc.gpsimd.dma_start(wv_sb[e], io["wv"][sl, :])
                for e in range(NE):
                    sl = slice(P * e, P * e + P)
                    nc.scalar.dma_start(vT_sb[e], io["vT"][sl, :])

                # --- heads 0-1 projections first, then V, then attention ---
                proj_kq(wk_sb, kT_sb, bk_sb, k_sb, 0)
                knorms(0)
                knorms(1)
                proj_kq(wq_sb, qT_sb, bq_sb, q_sb, 0)
                for h in (0, 1):
                    nc.sync.dma_start(q_sb[h][DK:DK + 1, :], io["ones_row"])
                # V projection must be emitted before the attention that
                # consumes it — Tile dependencies follow program order.
                for st in range(NST):
                    ssl = slice(P * st, P * st + P)
                    ps = psum.tile([P, VW], F32, tag="small", name="projv", bufs=4)
                    for e in range(NE):
                        nc.tensor.matmul(
                            ps, vT_sb[e][:, ssl], wv_sb[e], start=(e == 0), stop=False
                        )
                    nc.tensor.matmul(ps, ones_sb[:, 0:P], bv_sb, start=False, stop=True)
                    nc.vector.tensor_copy(v_sb[st], ps)
                attn_head(0)

            # head 1 attention next (its projections landed with head 0's);
            # heads 2-3 projections then fill PE slack during it.
            attn_head(1)
            proj_kq(wk_sb, kT_sb, bk_sb, k_sb, 1)
            knorms(2)
            knorms(3)
            proj_kq(wq_sb, qT_sb, bq_sb, q_sb, 1)
            for h in (2, 3):
                nc.sync.dma_start(q_sb[h][DK:DK + 1, :], io["ones_row"])
            attn_head(2)
            attn_head(3)

        # ---------------- Stage E: residual + LayerNorm ----------------
        with (
            tc.tile_pool(name="sres", bufs=NST) as sres,
            tc.tile_pool(name="sstat", bufs=8) as sstat,
            tc.tile_pool(name="sgrp", bufs=1) as sgrp,
            tc.tile_pool(name="sout", bufs=4) as sout,
        ):
            NG = 2  # AllReduce split for tail pipelining
            GST = NST // NG
            stats_in = [dram.tile([P, 2 * GST], F32, tag=f"stats_in{g}",
                                  name=f"stats_in{g}") for g in range(NG)]
            stats_out = [dram.tile([P, 2 * GST], F32, tag=f"stats_out{g}",
                                   name=f"stats_out{g}") for g in range(NG)]
            stats_sb = [sgrp.tile([P, 2 * GST], F32, tag=f"stats_sb{g}",
                                  name=f"stats_sb{g}") for g in range(NG)]
            gstats_sb = [sgrp.tile([P, 2 * GST], F32, tag=f"gstats_sb{g}",
                                   name=f"gstats_sb{g}") for g in range(NG)]
            for grp in range(NG):
                for sti in range(GST):
                    st = grp * GST + sti
                    ssl = slice(P * st, P * st + P)
                    r = sres.tile([P, DHC], F32, tag="res", name="res")
                    nc.sync.dma_start(r, io["resid"][ssl, :])
                    nc.vector.tensor_add(x_sb[st], x_sb[st], r)
                    s6 = sstat.tile([P, 6], F32, tag="s6", name="s6")
                    nc.vector.bn_stats(s6, x_sb[st])
                    mv = sstat.tile([P, 2], F32, tag="mv", name="mv")
                    nc.vector.bn_aggr(mv, s6)
                    # partial sums over this core's 256 features:
                    # [sum, sumsq] = [mean*256, (var+mean^2)*256]
                    nc.vector.tensor_scalar_mul(
                        stats_sb[grp][:, 2 * sti:2 * sti + 1], mv[:, 0:1], float(DHC)
                    )
                    t1 = sstat.tile([P, 1], F32, tag="t1", name="t1")
                    nc.vector.tensor_mul(t1, mv[:, 0:1], mv[:, 0:1])
                    nc.vector.tensor_add(t1, t1, mv[:, 1:2])
                    nc.vector.tensor_scalar_mul(
                        stats_sb[grp][:, 2 * sti + 1:2 * sti + 2], t1, float(DHC)
                    )
                nc.sync.dma_start(stats_in[grp][:, :], stats_sb[grp])

                if no_collective:
                    nc.sync.dma_start(stats_out[grp][:, :], stats_in[grp][:, :])
                else:
                    nc.gpsimd.collective_compute(
                        "AllReduce",
                        mybir.AluOpType.add,
                        replica_groups=[[0, 1, 2, 3], [4, 5, 6, 7]],
                        ins=[stats_in[grp].opt()],
                        outs=[stats_out[grp].opt()],
                    )
                nc.sync.dma_start(gstats_sb[grp], stats_out[grp][:, :])

            inv_n1 = 1.0 / float(E - 1)
            for st in range(NST):
                ssl = slice(P * st, P * st + P)
                grp, sti = st // GST, st % GST
                g = gstats_sb[grp][:, 2 * sti:2 * sti + 2]
                mean = sstat.tile([P, 1], F32, tag="mean", name="mean")
                nc.vector.tensor_scalar_mul(mean, g[:, 0:1], 1.0 / float(E))
                m2 = sstat.tile([P, 1], F32, tag="m2", name="m2")
                nc.vector.tensor_mul(m2, mean, mean)
                nc.vector.tensor_scalar_mul(m2, m2, float(E) * inv_n1)
                var = sstat.tile([P, 1], F32, tag="var", name="var")
                nc.vector.tensor_scalar_mul(var, g[:, 1:2], inv_n1)
                nc.vector.tensor_sub(var, var, m2)
                stddev = sstat.tile([P, 1], F32, tag="std", name="std")
                nc.scalar.activation(stddev, var, AF.Sqrt, bias=0.0, scale=1.0)
                o = sout.tile([P, DHC], F32, tag="o", name="o")
                if ln_fast:
                    # shift==0, scale==1: div is per-row -> single fused op.
                    rdiv = sstat.tile([P, 1], F32, tag="rdiv", name="rdiv")
                    nc.vector.tensor_scalar_add(stddev, stddev, float(EPS))
                    nc.vector.reciprocal(rdiv, stddev)
                    nc.vector.tensor_scalar(
                        o, x_sb[st], mean, rdiv,
                        op0=mybir.AluOpType.subtract, op1=mybir.AluOpType.mult,
                    )
                else:
                    div = sout.tile([P, DHC], F32, tag="div", name="div")
                    nc.vector.tensor_scalar_add(div, esb, stddev)
                    rdiv = sout.tile([P, DHC], F32, tag="rdiv", name="rdiv")
                    nc.vector.reciprocal(rdiv, div)
                    xm = sout.tile([P, DHC], F32, tag="xm", name="xm")
                    nc.vector.tensor_scalar_sub(xm, x_sb[st], mean)
                    nc.vector.tensor_mul(xm, xm, rdiv)
                    nc.vector.tensor_mul(o, xm, lnsc)
                oeng = [nc.scalar, nc.sync][st % 2]
                oeng.dma_start(io["out"][ssl, :], o)


def build_nc(n_reps=1, ln_fast=False):
    global _NC_CACHE
    cache_key = (n_reps, ln_fast)
    if _NC_CACHE is not None and _NC_CACHE[0] == cache_key:
        return _NC_CACHE[1]
    nc = bacc.Bacc(
        "TRN2",
        target_bir_lowering=False,
        debug=False,
        enable_asserts=True,
        num_devices=N_CORES,
    )
    io = {
        "qT": nc.dram_tensor("qT", [E, S], BF16, kind="ExternalInput").ap(),
        "kT": nc.dram_tensor("kT", [E, S], BF16, kind="ExternalInput").ap(),
        "vT": nc.dram_tensor("vT", [E, S], BF16, kind="ExternalInput").ap(),
        "wq": nc.dram_tensor("wq", [E, DHC], BF16, kind="ExternalInput").ap(),
        "bq": nc.dram_tensor("bq", [1, DHC], BF16, kind="ExternalInput").ap(),
        "wk": nc.dram_tensor("wk", [E, DHC], BF16, kind="ExternalInput").ap(),
        "bk": nc.dram_tensor("bk", [1, DHC], BF16, kind="ExternalInput").ap(),
        "wv": nc.dram_tensor("wv", [E, VW], BF16, kind="ExternalInput").ap(),
        "bv": nc.dram_tensor("bv", [1, VW], BF16, kind="ExternalInput").ap(),
        "ones_row": nc.dram_tensor("ones_row", [1, S], BF16, kind="ExternalInput").ap(),
        "resid": nc.dram_tensor("resid", [S, DHC], F32, kind="ExternalInput").ap(),
        "lnscale": nc.dram_tensor("lnscale", [1, DHC], F32, kind="ExternalInput").ap(),
        "epsshift": nc.dram_tensor("epsshift", [1, DHC], F32, kind="ExternalInput").ap(),
        "out": nc.dram_tensor("out", [S, DHC], F32, kind="ExternalOutput").ap(),
    }
    with tile.TileContext(nc) as tc:
        for _ in range(n_reps):
            _emit(nc, tc, io, ln_fast=ln_fast)
    nc.compile()
    _NC_CACHE = (cache_key, nc)
    return nc


def prep_inputs(query, key, value, residual_x, Wq, bq, Wk, bk, Wv, bv, scale, shift):
    query = np.asarray(query)
    key = np.asarray(key)
    value = np.asarray(value)
    residual_x = np.asarray(residual_x)
    Wq = np.asarray(Wq)
    bq = np.asarray(bq)
    Wk = np.asarray(Wk)
    bk = np.asarray(bk)
    Wv = np.asarray(Wv)
    bv = np.asarray(bv)
    scale = np.asarray(scale)
    shift = np.asarray(shift)

    ones_row = np.ones((1, S), NPBF16)
    perb = []
    for b in range(B):
        perb.append(
            dict(
                qT=np.ascontiguousarray(query[b].T).astype(NPBF16),
                kT=np.ascontiguousarray(key[b].T).astype(NPBF16),
                vT=np.ascontiguousarray(value[b].T).astype(NPBF16),
            )
        )
    in_maps = []
    for c in range(N_CORES):
        b = c // 4
        gidx = c % 4
        h0 = HPC * gidx
        fsl = slice(DHC * gidx, DHC * gidx + DHC)
        wq = (Wq[h0:h0 + HPC] * SCALE).transpose(1, 0, 2).reshape(E, DHC)
        bq_ = (bq[h0:h0 + HPC] * SCALE).reshape(1, DHC)
        wk = Wk[h0:h0 + HPC].transpose(1, 0, 2).reshape(E, DHC)
        bk_ = bk[h0:h0 + HPC].reshape(1, DHC)
        wv = np.zeros((E, VW), np.float32)
        bv_ = np.zeros((1, VW), np.float32)
        for h in range(HPC):
            wv[:, (DV + 1) * h:(DV + 1) * h + DV] = Wv[h0 + h]
            bv_[0, (DV + 1) * h:(DV + 1) * h + DV] = bv[h0 + h]
            bv_[0, (DV + 1) * h + DV] = 1.0
        in_maps.append(
            dict(
                qT=perb[b]["qT"],
                kT=perb[b]["kT"],
                vT=perb[b]["vT"],
                wq=wq.astype(NPBF16),
                bq=bq_.astype(NPBF16),
                wk=wk.astype(NPBF16),
                bk=bk_.astype(NPBF16),
                wv=wv.astype(NPBF16),
                bv=bv_.astype(NPBF16),
                ones_row=ones_row,
                resid=np.ascontiguousarray(residual_x[b][:, fsl]).astype(np.float32),
                lnscale=np.ascontiguousarray(scale[fsl]).reshape(1, DHC).astype(np.float32),
                epsshift=(EPS + shift[fsl]).reshape(1, DHC).astype(np.float32),
            )
        )
    return in_maps


def assemble_output(results):
    out = np.empty((B, S, E), np.float32)
    for c in range(N_CORES):
        b = c // 4
        gidx = c % 4
        out[b, :, DHC * gidx:DHC * gidx + DHC] = results[c]["out"]
    return out


def ln_fast_ok(scale, shift):
    scale = np.asarray(scale)
    shift = np.asarray(shift)
    return bool(np.all(shift == 0.0) and np.all(scale == 1.0))


def kernel(**inputs):
    nc = build_nc(ln_fast=ln_fast_ok(inputs["scale"], inputs["shift"]))
    in_maps = prep_inputs(**inputs)
    res = bass_utils.run_bass_kernel_spmd(
        nc, in_maps, core_ids=list(range(N_CORES))
    )
    return assemble_output(res.results)


# revision 43
# speedup vs baseline: 1.4408x; 1.4408x over previous
"""Trainium2 Bass kernel for nn_MultiHeadAttention_77283641524724.

Gaussian-kernel multi-head attention + residual + custom LayerNorm.

Sharding (8 cores): core c handles batch c//4 and heads [4*(c%4), 4*(c%4)+4).
Each core computes its 4 heads' QKV projections, attention, and its 256-col
slice of the head-concat; LayerNorm (over the full 1024 features) is realized
with a tiny AllReduce of per-row partial (sum, sumsq) within each batch's
4-core group, after which every core normalizes its own feature slice.
Host-side gather is a plain concatenate along the feature axis.

Math notes:
- scores = scale*(q.k - 0.5||q||^2 - 0.5||k||^2); the -0.5||q||^2 term is a
  per-query-row constant and softmax is invariant to it -> dropped.
- scale is folded into Wq/bq on the host.
- -0.5*scale*||k||^2 rides in the score matmul as a 65th contraction row
  (k-side row = norms, q-side row = ones).
- score range is ~[-0.7, 0.4] for this distribution -> exp without
  max-subtraction is safe (reference softmax is shift-invariant).
- softmax denominator comes from a ones-column appended to V (65-col
  stationary operand), so attn@V yields [out | norm] in one accumulation.
- matmul operands are bf16 (fp32 PSUM accumulation); residual add + LN are
  fp32.
"""

import numpy as np
import ml_dtypes

import concourse.bass as bass
import concourse.bacc as bacc
import concourse.tile as tile
from concourse import mybir
import concourse.bass_utils as bass_utils
from concourse.masks import make_identity

BF16 = mybir.dt.bfloat16
F32 = mybir.dt.float32
NPBF16 = ml_dtypes.bfloat16

B, S, E = 2, 2048, 1024
H, DK, DV = 16, 64, 64
EPS = 1e-6
SCALE = 1.0 / float(np.sqrt(np.float32(E)))
N_CORES = 8
HPC = 4            # heads per core
DHC = HPC * DV     # 256 output cols per core
VW = HPC * (DV + 1)  # 260: v + ones col per head
P = 128
NE = E // P        # 8 contraction tiles
NST = S // P       # 16 seq tiles of 128
NSC = S // 512     # 4 seq chunks of 512
NKT = S // P       # 16 key tiles
AF = mybir.ActivationFunctionType

_NC_CACHE = None


def _bcast_ap(ap, p):
    """[1, n] DRAM AP -> [[0, p], [1, n]] partition-broadcast AP."""
    return bass.AP(tensor=ap.tensor, offset=ap.offset, ap=[[0, p], ap.ap[-1]])


def _emit(nc, tc, io, no_collective=False, ln_fast=False):
    from contextlib import ExitStack

    with ExitStack() as ctx:
        consts = ctx.enter_context(tc.tile_pool(name="consts", bufs=1))
        persist = ctx.enter_context(tc.tile_pool(name="persist", bufs=1))
        dram = ctx.enter_context(tc.tile_pool(name="dram", bufs=1, space="DRAM"))

        ident = consts.tile([P, P], F32, tag="ident", name="ident")
        make_identity(nc, ident)
        negcol = consts.tile([DK, 1], BF16, tag="negcol", name="negcol")
        nc.vector.memset(negcol, -0.5 * SCALE)
        # Small consts ride the SWDGE queue so the HWDGE queues start on the
        # big input tensors immediately.
        ones_sb = consts.tile([1, S], BF16, tag="ones", name="ones")
        nc.gpsimd.dma_start(ones_sb, io["ones_row"])
        bq_sb = consts.tile([1, DHC], BF16, tag="bq", name="bq")
        nc.gpsimd.dma_start(bq_sb, io["bq"])
        bk_sb = consts.tile([1, DHC], BF16, tag="bk", name="bk")
        nc.gpsimd.dma_start(bk_sb, io["bk"])
        bv_sb = consts.tile([1, VW], BF16, tag="bv", name="bv")
        nc.gpsimd.dma_start(bv_sb, io["bv"])
        esb = consts.tile([P, DHC], F32, tag="esb", name="esb")
        nc.gpsimd.dma_start(esb, _bcast_ap(io["epsshift"], P))
        lnsc = consts.tile([P, DHC], F32, tag="lnsc", name="lnsc")
        nc.gpsimd.dma_start(lnsc, _bcast_ap(io["lnscale"], P))

        # Persistent per-head / per-seq-tile tensors.
        q_sb = [persist.tile([DK + 1, S], BF16, tag=f"q{h}", name=f"q{h}") for h in range(HPC)]
        k_sb = [persist.tile([DK + 1, S], BF16, tag=f"k{h}", name=f"k{h}") for h in range(HPC)]
        v_sb = [persist.tile([P, VW], BF16, tag=f"v{st}", name=f"v{st}") for st in range(NST)]
        x_sb = [persist.tile([P, DHC], F32, tag=f"x{st}", name=f"x{st}") for st in range(NST)]

        # ------- Stages B (projections) + D (attention), interleaved -------
        # Emission order sets scheduler priority: V; K/Q for heads 0-1;
        # attention heads 0-1; K/Q for heads 2-3 (fills PE while ACT crunches
        # exp); attention heads 2-3.  One shared PSUM pool: "scores" 2x2
        # banks + "small" 4x1 banks (projections / attn accumulators /
        # transposes all fit a [128,512]-f32 slot).
        with (
            tc.tile_pool(name="kqin", bufs=1) as kqin,
            tc.tile_pool(name="psum", bufs=1, space="PSUM") as psum,
            tc.tile_pool(name="sksq", bufs=1) as sksq,
            tc.tile_pool(name="sexp", bufs=12) as sexp,
            tc.tile_pool(name="susb", bufs=3) as susb,
            tc.tile_pool(name="ssml", bufs=8) as ssml,
        ):
            kT_sb = [kqin.tile([P, S], BF16, tag=f"kT{e}", name=f"kT{e}") for e in range(NE)]
            qT_sb = [kqin.tile([P, S], BF16, tag=f"qT{e}", name=f"qT{e}") for e in range(NE)]
            wq_sb = [kqin.tile([P, DHC], BF16, tag=f"wq{e}", name=f"wq{e}") for e in range(NE)]
            wk_sb = [kqin.tile([P, DHC], BF16, tag=f"wk{e}", name=f"wk{e}") for e in range(NE)]

            def proj_chunk(w_sb, in_sb, b_row, dst, dt, sc, norms=False):
                # dst[2dt], dst[2dt+1] rows 0:64 <- [d, s-chunk] projection;
                # with norms=True also fills k-norm row 64 for this chunk.
                dsl = slice(P * dt, P * dt + P)
                ssl = slice(512 * sc, 512 * sc + 512)
                ps = psum.tile([P, 512], F32, tag="small", name="proj", bufs=4)
                for e in range(NE):
                    nc.tensor.matmul(
                        ps, w_sb[e][:, dsl], in_sb[e][:, ssl],
                        start=(e == 0), stop=False,
                    )
                nc.tensor.matmul(
                    ps, b_row[:, dsl], ones_sb[:, ssl],
                    start=False, stop=True,
                )
                nc.vector.tensor_copy(dst[2 * dt][0:DK, ssl], ps[0:DK, :])
                nc.vector.tensor_copy(dst[2 * dt + 1][0:DK, ssl], ps[DK:P, :])
                if norms:
                    for h in (2 * dt, 2 * dt + 1):
                        ksq = sksq.tile([DK, 512], BF16, tag="ksq", name="ksq",
                                        bufs=3)
                        nc.vector.tensor_mul(ksq, k_sb[h][0:DK, ssl],
                                             k_sb[h][0:DK, ssl])
                        pn = psum.tile([1, 512], F32, tag="small", name="pn",
                                       bufs=4)
                        nc.tensor.matmul(pn, negcol, ksq, start=True, stop=True)
                        nc.vector.tensor_copy(k_sb[h][DK:DK + 1, ssl], pn)

            def proj_kq(w_sb, in_sb, b_row, dst, dt, norms=False):
                for sc in range(NSC):
                    proj_chunk(w_sb, in_sb, b_row, dst, dt, sc, norms=norms)

            def attn_head(h):
                vsl = slice((DV + 1) * h, (DV + 1) * (h + 1))
                for qh in range(2):  # query halves of 1024
                    avs = [psum.tile([DV + 1, 512], F32, tag="small", name="av", bufs=4)
                           for _ in range(2)]
                    for kt in range(NKT):
                        ksl = slice(P * kt, P * kt + P)
                        sc_ps = psum.tile([P, 1024], F32, tag="scores", name="scores", bufs=2)
                        for qq in range(2):
                            qc = 2 * qh + qq
                            qsl = slice(512 * qc, 512 * qc + 512)
                            nc.tensor.matmul(
                                sc_ps[:, 512 * qq:512 * qq + 512],
                                k_sb[h][:, ksl], q_sb[h][:, qsl],
                                start=True, stop=True,
                            )
                        e_sb = sexp.tile([P, 1024], BF16, tag="exp", name="exp")
                        nc.scalar.activation(e_sb, sc_ps, AF.Exp, bias=0.0, scale=1.0)
                        for qq in range(2):
                            nc.tensor.matmul(
                                avs[qq], v_sb[kt][:, vsl],
                                e_sb[:, 512 * qq:512 * qq + 512],
                                start=(kt == 0), stop=(kt == NKT - 1),
                            )
                    for qq in range(2):
                        u = susb.tile([DV + 1, 512], F32, tag="usb", name="usb")
                        nc.vector.tensor_copy(u, avs[qq])
                        for pi in range(4):
                            st = 8 * qh + 4 * qq + pi
                            tp = psum.tile([P, DV + 1], F32, tag="small", name="tp", bufs=4)
                            nc.tensor.transpose(
                                tp, u[:, P * pi:P * pi + P],
                                ident[0:DV + 1, 0:DV + 1],
                            )
                            rec = ssml.tile([P, 1], F32, tag="rec", name="rec")
                            nc.vector.reciprocal(rec, tp[:, DV:DV + 1])
                            nc.vector.tensor_scalar_mul(
                                x_sb[st][:, DV * h:DV * h + DV], tp[:, 0:DV], rec
                            )

            with tc.tile_pool(name="vin", bufs=1) as vin:
                vT_sb = [vin.tile([P, S], BF16, tag=f"vT{e}", name=f"vT{e}") for e in range(NE)]
                wv_sb = [vin.tile([P, VW], BF16, tag=f"wv{e}", name=f"wv{e}") for e in range(NE)]
                # Three DMA queues in parallel: SP=wv/vT, ACT=wk/kT,
                # SWDGE=wq/qT.  K/Q gate the first scores; V matmuls are
                # pulled in by the scheduler per k-tile as attention needs
                # them.
                for e in range(NE):
                    sl = slice(P * e, P * e + P)
                    nc.sync.dma_start(wq_sb[e], io["wq"][sl, :])
                    nc.sync.dma_start(qT_sb[e], io["qT"][sl, :])
                    nc.scalar.dma_start(wk_sb[e], io["wk"][sl, :])
                    nc.scalar.dma_start(kT_sb[e], io["kT"][sl, :])
                    nc.gpsimd.dma_start(wv_sb[e], io["wv"][sl, :])
                for e in range(NE):
                    sl = slice(P * e, P * e + P)
                    nc.scalar.dma_start(vT_sb[e], io["vT"][sl, :])

                # --- heads 0-1 projections first (K/Q chunk-interleaved so
                # the first scores fire early), then V, then attention ---
                for sc in range(NSC):
                    proj_chunk(wk_sb, kT_sb, bk_sb, k_sb, 0, sc, norms=True)
                    proj_chunk(wq_sb, qT_sb, bq_sb, q_sb, 0, sc)
                for h in (0, 1):
                    nc.sync.dma_start(q_sb[h][DK:DK + 1, :], io["ones_row"])
                # V projection must be emitted before the attention that
                # consumes it — Tile dependencies follow program order.
                for st in range(NST):
                    ssl = slice(P * st, P * st + P)
                    ps = psum.tile([P, VW], F32, tag="small", name="projv", bufs=4)
                    for e in range(NE):
                        nc.tensor.matmul(
                            ps, vT_sb[e][:, ssl], wv_sb[e], start=(e == 0), stop=False
                        )
                    nc.tensor.matmul(ps, ones_sb[:, 0:P], bv_sb, start=False, stop=True)
                    nc.vector.tensor_copy(v_sb[st], ps)
                attn_head(0)

            # head 1 attention next (its projections landed with head 0's);
            # heads 2-3 projections then fill PE slack during it.
            attn_head(1)
            for sc in range(NSC):
                proj_chunk(wk_sb, kT_sb, bk_sb, k_sb, 1, sc, norms=True)
                proj_chunk(wq_sb, qT_sb, bq_sb, q_sb, 1, sc)
            for h in (2, 3):
                nc.sync.dma_start(q_sb[h][DK:DK + 1, :], io["ones_row"])
            attn_head(2)
            attn_head(3)

        # ---------------- Stage E: residual + LayerNorm ----------------
        with (
            tc.tile_pool(name="sres", bufs=NST) as sres,
            tc.tile_pool(name="sstat", bufs=8) as sstat,
            tc.tile_pool(name="sgrp", bufs=1) as sgrp,
            tc.tile_pool(name="sout", bufs=4) as sout,
        ):
            NG = 2  # AllReduce split for tail pipelining
            GST = NST // NG
            stats_in = [dram.tile([P, 2 * GST], F32, tag=f"stats_in{g}",
                                  name=f"stats_in{g}") for g in range(NG)]
            stats_out = [dram.tile([P, 2 * GST], F32, tag=f"stats_out{g}",
                                   name=f"stats_out{g}") for g in range(NG)]
            stats_sb = [sgrp.tile([P, 2 * GST], F32, tag=f"stats_sb{g}",
                                  name=f"stats_sb{g}") for g in range(NG)]
            gstats_sb = [sgrp.tile([P, 2 * GST], F32, tag=f"gstats_sb{g}",
                                   name=f"gstats_sb{g}") for g in range(NG)]
            for grp in range(NG):
                for sti in range(GST):
                    st = grp * GST + sti
                    ssl = slice(P * st, P * st + P)
                    r = sres.tile([P, DHC], F32, tag="res", name="res")
                    nc.sync.dma_start(r, io["resid"][ssl, :])
                    nc.vector.tensor_add(x_sb[st], x_sb[st], r)
                    s6 = sstat.tile([P, 6], F32, tag="s6", name="s6")
                    nc.vector.bn_stats(s6, x_sb[st])
                    mv = sstat.tile([P, 2], F32, tag="mv", name="mv")
                    nc.vector.bn_aggr(mv, s6)
                    # partial sums over this core's 256 features:
                    # [sum, sumsq] = [mean*256, (var+mean^2)*256]
                    nc.vector.tensor_scalar_mul(
                        stats_sb[grp][:, 2 * sti:2 * sti + 1], mv[:, 0:1], float(DHC)
                    )
                    t1 = sstat.tile([P, 1], F32, tag="t1", name="t1")
                    nc.vector.tensor_mul(t1, mv[:, 0:1], mv[:, 0:1])
                    nc.vector.tensor_add(t1, t1, mv[:, 1:2])
                    nc.vector.tensor_scalar_mul(
                        stats_sb[grp][:, 2 * sti + 1:2 * sti + 2], t1, float(DHC)
                    )
                nc.sync.dma_start(stats_in[grp][:, :], stats_sb[grp])

                if no_collective:
                    nc.sync.dma_start(stats_out[grp][:, :], stats_in[grp][:, :])
                else:
                    nc.gpsimd.collective_compute(
                        "AllReduce",
                        mybir.AluOpType.add,
                        replica_groups=[[0, 1, 2, 3], [4, 5, 6, 7]],
                        ins=[stats_in[grp].opt()],
                        outs=[stats_out[grp].opt()],
                    )
                nc.sync.dma_start(gstats_sb[grp], stats_out[grp][:, :])

            inv_n1 = 1.0 / float(E - 1)
            for st in range(NST):
                ssl = slice(P * st, P * st + P)
                grp, sti = st // GST, st % GST
                g = gstats_sb[grp][:, 2 * sti:2 * sti + 2]
                mean = sstat.tile([P, 1], F32, tag="mean", name="mean")
                nc.vector.tensor_scalar_mul(mean, g[:, 0:1], 1.0 / float(E))
                m2 = sstat.tile([P, 1], F32, tag="m2", name="m2")
                nc.vector.tensor_mul(m2, mean, mean)
                nc.vector.tensor_scalar_mul(m2, m2, float(E) * inv_n1)
                var = sstat.tile([P, 1], F32, tag="var", name="var")
                nc.vector.tensor_scalar_mul(var, g[:, 1:2], inv_n1)
                nc.vector.tensor_sub(var, var, m2)
                stddev = sstat.tile([P, 1], F32, tag="std", name="std")
                nc.scalar.activation(stddev, var, AF.Sqrt, bias=0.0, scale=1.0)
                o = sout.tile([P, DHC], F32, tag="o", name="o")
                if ln_fast:
                    # shift==0, scale==1: div is per-row -> single fused op.
                    rdiv = sstat.tile([P, 1], F32, tag="rdiv", name="rdiv")
                    nc.vector.tensor_scalar_add(stddev, stddev, float(EPS))
                    nc.vector.reciprocal(rdiv, stddev)
                    nc.vector.tensor_scalar(
                        o, x_sb[st], mean, rdiv,
                        op0=mybir.AluOpType.subtract, op1=mybir.AluOpType.mult,
                    )
                else:
                    div = sout.tile([P, DHC], F32, tag="div", name="div")
                    nc.vector.tensor_scalar_add(div, esb, stddev)
                    rdiv = sout.tile([P, DHC], F32, tag="rdiv", name="rdiv")
                    nc.vector.reciprocal(rdiv, div)
                    xm = sout.tile([P, DHC], F32, tag="xm", name="xm")
                    nc.vector.tensor_scalar_sub(xm, x_sb[st], mean)
                    nc.vector.tensor_mul(xm, xm, rdiv)
                    nc.vector.tensor_mul(o, xm, lnsc)
                oeng = [nc.scalar, nc.sync][st % 2]
                oeng.dma_start(io["out"][ssl, :], o)


def build_nc(n_reps=1, ln_fast=False):
    global _NC_CACHE
    cache_key = (n_reps, ln_fast)
    if _NC_CACHE is not None and _NC_CACHE[0] == cache_key:
        return _NC_CACHE[1]
    nc = bacc.Bacc(
        "TRN2",
        target_bir_lowering=False,
        debug=False,
        enable_asserts=True,
        num_devices=N_CORES,
    )
    io = {
        "qT": nc.dram_tensor("qT", [E, S], BF16, kind="ExternalInput").ap(),
        "kT": nc.dram_tensor("kT", [E, S], BF16, kind="ExternalInput").ap(),
        "vT": nc.dram_tensor("vT", [E, S], BF16, kind="ExternalInput").ap(),
        "wq": nc.dram_tensor("wq", [E, DHC], BF16, kind="ExternalInput").ap(),
        "bq": nc.dram_tensor("bq", [1, DHC], BF16, kind="ExternalInput").ap(),
        "wk": nc.dram_tensor("wk", [E, DHC], BF16, kind="ExternalInput").ap(),
        "bk": nc.dram_tensor("bk", [1, DHC], BF16, kind="ExternalInput").ap(),
        "wv": nc.dram_tensor("wv", [E, VW], BF16, kind="ExternalInput").ap(),
        "bv": nc.dram_tensor("bv", [1, VW], BF16, kind="ExternalInput").ap(),
        "ones_row": nc.dram_tensor("ones_row", [1, S], BF16, kind="ExternalInput").ap(),
        "resid": nc.dram_tensor("resid", [S, DHC], F32, kind="ExternalInput").ap(),
        "lnscale": nc.dram_tensor("lnscale", [1, DHC], F32, kind="ExternalInput").ap(),
        "epsshift": nc.dram_tensor("epsshift", [1, DHC], F32, kind="ExternalInput").ap(),
        "out": nc.dram_tensor("out", [S, DHC], F32, kind="ExternalOutput").ap(),
    }
    with tile.TileContext(nc) as tc:
        for _ in range(n_reps):
            _emit(nc, tc, io, ln_fast=ln_fast)
    nc.compile()
    _NC_CACHE = (cache_key, nc)
    return nc


def prep_inputs(query, key, value, residual_x, Wq, bq, Wk, bk, Wv, bv, scale, shift):
    query = np.asarray(query)
    key = np.asarray(key)
    value = np.asarray(value)
    residual_x = np.asarray(residual_x)
    Wq = np.asarray(Wq)
    bq = np.asarray(bq)
    Wk = np.asarray(Wk)
    bk = np.asarray(bk)
    Wv = np.asarray(Wv)
    bv = np.asarray(bv)
    scale = np.asarray(scale)
    shift = np.asarray(shift)

    ones_row = np.ones((1, S), NPBF16)
    perb = []
    for b in range(B):
        perb.append(
            dict(
                qT=np.ascontiguousarray(query[b].T).astype(NPBF16),
                kT=np.ascontiguousarray(key[b].T).astype(NPBF16),
                vT=np.ascontiguousarray(value[b].T).astype(NPBF16),
            )
        )
    in_maps = []
    for c in range(N_CORES):
        b = c // 4
        gidx = c % 4
        h0 = HPC * gidx
        fsl = slice(DHC * gidx, DHC * gidx + DHC)
        wq = (Wq[h0:h0 + HPC] * SCALE).transpose(1, 0, 2).reshape(E, DHC)
        bq_ = (bq[h0:h0 + HPC] * SCALE).reshape(1, DHC)
        wk = Wk[h0:h0 + HPC].transpose(1, 0, 2).reshape(E, DHC)
        bk_ = bk[h0:h0 + HPC].reshape(1, DHC)
        wv = np.zeros((E, VW), np.float32)
        bv_ = np.zeros((1, VW), np.float32)
        for h in range(HPC):
            wv[:, (DV + 1) * h:(DV + 1) * h + DV] = Wv[h0 + h]
            bv_[0, (DV + 1) * h:(DV + 1) * h + DV] = bv[h0 + h]
            bv_[0, (DV + 1) * h + DV] = 1.0
        in_maps.append(
            dict(
                qT=perb[b]["qT"],
                kT=perb[b]["kT"],
                vT=perb[b]["vT"],
                wq=wq.astype(NPBF16),
                bq=bq_.astype(NPBF16),
                wk=wk.astype(NPBF16),
                bk=bk_.astype(NPBF16),
                wv=wv.astype(NPBF16),
                bv=bv_.astype(NPBF16),
                ones_row=ones_row,
                resid=np.ascontiguousarray(residual_x[b][:, fsl]).astype(np.float32),
                lnscale=np.ascontiguousarray(scale[fsl]).reshape(1, DHC).astype(np.float32),
                epsshift=(EPS + shift[fsl]).reshape(1, DHC).astype(np.float32),
            )
        )
    return in_maps


def assemble_output(results):
    out = np.empty((B, S, E), np.float32)
    for c in range(N_CORES):
        b = c // 4
        gidx = c % 4
        out[b, :, DHC * gidx:DHC * gidx + DHC] = results[c]["out"]
    return out


def ln_fast_ok(scale, shift):
    scale = np.asarray(scale)
    shift = np.asarray(shift)
    return bool(np.all(shift == 0.0) and np.all(scale == 1.0))


def kernel(**inputs):
    nc = build_nc(ln_fast=ln_fast_ok(inputs["scale"], inputs["shift"]))
    in_maps = prep_inputs(**inputs)
    res = bass_utils.run_bass_kernel_spmd(
        nc, in_maps, core_ids=list(range(N_CORES))
    )
    return assemble_output(res.results)
